# revision 22
# baseline (speedup 1.0000x reference)
"""Trainium2 Bass kernel for nn_MultiHeadAttention (B=2, C=1024, H=16, S=2048).

Sharding: 8 cores = 2 batches x 4 head-groups (4 heads per core).
Per core:
  - QKV projections for its 4 heads (column-sharded weights), bias folded in
    via a ones-row appended to the inputs (K=1025 contraction).
  - Scores computed TRANSPOSED (scoresT[j,i] = k.q) so softmax normalization
    can ride along the ctx matmul: vT is augmented with a ones column so the
    ctx PSUM accumulates both sum_j P[j,i]*v[j,d] and sum_j P[j,i] (denom).
  - exp on ACT (no max-subtract needed: |scores| <= ~3), mask applied
    multiplicatively AFTER exp (exp(-1e9) == 0 equivalence).
  - Host does the final divide by the denominator row + transpose/concat.
"""

import numpy as np
import ml_dtypes

import concourse.bass as bass
import concourse.mybir as mybir
import concourse.tile as tile
from concourse import bacc
from concourse.bass_utils import run_bass_kernel_spmd

B = 2
C = 1024
HEADS = 16
CPH = 64
S = 2048
N_CORES = 8
HPC = 4  # heads per core
CPC = HPC * CPH  # channels per core = 256

BF = mybir.dt.bfloat16
F32 = mybir.dt.float32
F32R = mybir.dt.float32r
EXP = mybir.ActivationFunctionType.Exp

NBF = ml_dtypes.bfloat16

_NC_CACHE = {}


def build_nc():
    nc = bacc.Bacc("TRN2", target_bir_lowering=False)

    Qd = nc.declare_dram_parameter("Qin", [C, S], BF, isOutput=False)
    Kd = nc.declare_dram_parameter("Kin", [C, S], BF, isOutput=False)
    Vd = nc.declare_dram_parameter("Vin", [C, S], BF, isOutput=False)
    WqTd = nc.declare_dram_parameter("WqT", [C, CPC], BF, isOutput=False)
    WkTd = nc.declare_dram_parameter("WkT", [C, CPC], BF, isOutput=False)
    WvTd = nc.declare_dram_parameter("WvT", [C, HPC * 65], BF, isOutput=False)
    bqkd = nc.declare_dram_parameter("bqk", [128, 4], F32, isOutput=False)
    bvbd = nc.declare_dram_parameter("bvb", [128, HPC * 65], F32, isOutput=False)
    Md = nc.declare_dram_parameter("maskT", [S, S], BF, isOutput=False)
    Od = nc.declare_dram_parameter("out", [HPC * 65, S], F32, isOutput=True)

    with tile.TileContext(nc) as tc:
        with (
            tc.tile_pool(name="w", bufs=1) as wp,
            tc.tile_pool(name="qksb", bufs=1) as qkp,
            tc.tile_pool(name="vt", bufs=1) as vtp,
        ):
            # --- persistent SBUF tensors ---
            WqT = wp.tile([128, 8, CPC], BF, tag="wq")
            WkT = wp.tile([128, 8, CPC], BF, tag="wk")
            WvT = wp.tile([128, 8, HPC * 65], BF, tag="wv")
            bqk = wp.tile([128, 4], F32, tag="bqk")
            bvb = wp.tile([128, HPC * 65], F32, tag="bvb")
            for wt, wd in ((WqT, WqTd), (WkT, WkTd), (WvT, WvTd)):
                nc.sync.dma_start(
                    wt[:], wd[:].rearrange("(t p) n -> p t n", p=128)
                )
            nc.sync.dma_start(bqk[:], bqkd[:])
            nc.sync.dma_start(bvb[:], bvbd[:])

            q_sb = qkp.tile([128, 2, S], F32R, tag="q")  # pair-major, h-even rows 0:64
            k_sb = qkp.tile([128, 2, S], F32R, tag="k")
            vT = vtp.tile([128, 16, HPC * 65], BF, tag="vt")  # s_tile-major

            # --- phase 1: load inputs, projections (qk and vT interleaved) ---
            with tc.tile_pool(name="io", bufs=1) as io:
                Qin = io.tile([128, 8, S], BF, tag="qi")
                Kin = io.tile([128, 8, S], BF, tag="ki")
                Vin = io.tile([128, 8, S], BF, tag="vi")
                # V first (vT proj is first PE work), then Q/K interleaved
                for ci in range(8):
                    nc.sync.dma_start(Vin[:, ci, :], Vd[bass.ts(ci, 128), :])
                for ci in range(8):
                    nc.sync.dma_start(Qin[:, ci, :], Qd[bass.ts(ci, 128), :])
                    nc.sync.dma_start(Kin[:, ci, :], Kd[bass.ts(ci, 128), :])

                with (
                    tc.tile_pool(name="pp", bufs=6, space="PSUM") as pp,
                    tc.tile_pool(name="pv", bufs=2, space="PSUM") as pv,
                ):
                    # q/k projections per pair p, in i-halves (2-bank psum):
                    # psum rows 0:64 = head 2p, 64:128 = head 2p+1
                    # vT projection: vT[s, e] = sum_c V[c, s] * WvT[c, e]
                    for s in range(16):
                        ps = pv.tile([128, HPC * 65], F32, tag="pv")
                        for ci in range(8):
                            nc.tensor.matmul(
                                ps[:],
                                lhsT=Vin[:, ci, bass.ts(s, 128)],
                                rhs=WvT[:, ci, :],
                                start=(ci == 0),
                                stop=(ci == 7),
                            )
                        nc.vector.tensor_add(vT[:, s, :], ps[:], bvb[:])

                    for p in range(2):
                        for qk, (dst, wt, src) in enumerate(
                            ((q_sb, WqT, Qin), (k_sb, WkT, Kin))
                        ):
                            for n4 in range(4):
                                ps = pp.tile([128, 512], F32, tag="pp")
                                for ci in range(8):
                                    nc.tensor.matmul(
                                        ps[:],
                                        lhsT=wt[:, ci, bass.ts(p, 128)],
                                        rhs=src[:, ci, bass.ts(n4, 512)],
                                        start=(ci == 0),
                                        stop=(ci == 7),
                                    )
                                nc.scalar.add(
                                    dst[:, p, bass.ts(n4, 512)],
                                    ps[:],
                                    bqk[:, 2 * p + qk : 2 * p + qk + 1],
                                )

            # --- phase 3: attention ---
            with (
                tc.tile_pool(name="msk", bufs=1) as mkp,
                tc.tile_pool(name="pt", bufs=6) as ptp,
                tc.tile_pool(name="ob", bufs=4) as obp,
                tc.tile_pool(name="sc", bufs=2, space="PSUM") as scp,
                tc.tile_pool(name="cx", bufs=2, space="PSUM") as cxp,
            ):
                maskT = mkp.tile([128, 16, S], BF, tag="m")
                for j in range(16):
                    nc.sync.dma_start(
                        maskT[:, j, :], Md[bass.ts(j, 128), :]
                    )

                for p in range(2):
                    for hf in range(2):
                        cx = [
                            cxp.tile([65, 2, 512], F32, tag="cx", name=f"cx{i}")
                            for i in range(2)
                        ]
                        for j in range(16):
                            # both heads' score MMs emitted adjacently so the
                            # PE runs them concurrently (row groups 0-1 / 2-3)
                            sc0 = scp.tile([128, 1024], F32, tag="sc")
                            sc1 = scp.tile([128, 1024], F32, tag="sc")
                            for ib in range(2):
                                for hh, sc in ((0, sc0), (1, sc1)):
                                    lo, hi = 64 * hh, 64 * hh + 64
                                    nc.tensor.matmul(
                                        sc[:, bass.ts(ib, 512)],
                                        lhsT=k_sb[lo:hi, p, bass.ts(j, 128)],
                                        rhs=q_sb[
                                            lo:hi,
                                            p,
                                            bass.ds(hf * 1024 + ib * 512, 512),
                                        ],
                                        start=True,
                                        stop=True,
                                    )
                            pts = []
                            for hh, sc in ((0, sc0), (1, sc1)):
                                pt = ptp.tile([128, 1024], BF, tag="pt")
                                nc.scalar.activation(pt[:], sc[:], EXP)
                                nc.vector.tensor_mul(
                                    pt[:], pt[:], maskT[:, j, bass.ts(hf, 1024)]
                                )
                                pts.append(pt)
                            for hh in range(2):
                                hloc = 2 * p + hh
                                for ib in range(2):
                                    nc.tensor.matmul(
                                        cx[hh][:, ib, :],
                                        lhsT=vT[:, j, bass.ds(hloc * 65, 65)],
                                        rhs=pts[hh][:, bass.ts(ib, 512)],
                                        start=(j == 0),
                                        stop=(j == 15),
                                    )
                        # drain: each cx copy split across DVE+ACT so the
                        # banks release in ~0.6us and the next pass's ctx
                        # accumulation can start sooner
                        for hh in range(2):
                            hloc = 2 * p + hh
                            ob = obp.tile([65, 2, 512], F32, tag="ob")
                            nc.vector.tensor_copy(ob[:, 0, :], cx[hh][:, 0, :])
                            nc.scalar.copy(ob[:, 1, :], cx[hh][:, 1, :])
                            nc.sync.dma_start(
                                Od[
                                    bass.ds(hloc * 65, 65), bass.ts(hf, 1024)
                                ].rearrange("p (x y) -> p x y", x=2),
                                ob[:],
                            )
    nc.compile()
    return nc


def _get_nc():
    if "nc" not in _NC_CACHE:
        _NC_CACHE["nc"] = build_nc()
    return _NC_CACHE["nc"]


def _make_in_maps(Q, K, V, mask, Wq, bq, Wk, bk, Wv, bv):
    per_batch = []
    for b in range(B):
        Qa = Q[b].astype(NBF)
        Ka = K[b].astype(NBF)
        Va = V[b].astype(NBF)
        mT = np.ascontiguousarray((~mask[b]).T).astype(np.float32).astype(NBF)
        per_batch.append((Qa, Ka, Va, mT))

    in_maps = []
    for c in range(N_CORES):
        b, g = divmod(c, 4)
        hs = slice(g * CPC, (g + 1) * CPC)
        Qa, Ka, Va, mT = per_batch[b]
        WqTa = np.ascontiguousarray(Wq[hs].T / 8.0).astype(NBF)
        WkTa = np.ascontiguousarray(Wk[hs].T).astype(NBF)
        WvTa = np.zeros((C, HPC * 65), np.float32)
        bvba = np.zeros((128, HPC * 65), np.float32)
        for hh in range(HPC):
            ch = slice((g * HPC + hh) * CPH, (g * HPC + hh + 1) * CPH)
            WvTa[:, hh * 65 : hh * 65 + 64] = Wv[ch].T
            bvba[:, hh * 65 : hh * 65 + 64] = bv[ch][None, :]
            bvba[:, hh * 65 + 64] = 1.0
        # bias for q/k psum->sbuf copies: col 2p+qk = per-partition bias of
        # pair p's 128 channels (rows 0:64 = head 2p, 64:128 = head 2p+1)
        bqka = np.zeros((128, 4), np.float32)
        for p in range(2):
            ch = slice((g * 2 + p) * 128, (g * 2 + p + 1) * 128)
            bqka[:, 2 * p] = bq[ch] / 8.0
            bqka[:, 2 * p + 1] = bk[ch]
        in_maps.append(
            {
                "Qin": Qa,
                "Kin": Ka,
                "Vin": Va,
                "WqT": WqTa,
                "WkT": WkTa,
                "WvT": WvTa.astype(NBF),
                "bqk": bqka,
                "bvb": bvba,
                "maskT": mT,
            }
        )
    return in_maps


def _assemble(results):
    out = np.zeros((B, S, C), np.float32)
    for c in range(N_CORES):
        b, g = divmod(c, 4)
        o = results[c]["out"]  # [260, 2048]
        for hh in range(HPC):
            ctx = o[hh * 65 : hh * 65 + 64]  # [64, S] = (d, i)
            den = o[hh * 65 + 64]  # [S]
            ch0 = (g * HPC + hh) * CPH
            out[b, :, ch0 : ch0 + CPH] = (ctx / den[None, :]).T
    return out


def run(inputs, trace=False):
    in_maps = _make_in_maps(
        np.asarray(inputs["Q"], np.float32),
        np.asarray(inputs["K"], np.float32),
        np.asarray(inputs["V"], np.float32),
        np.asarray(inputs["mask"]),
        np.asarray(inputs["Wq"], np.float32),
        np.asarray(inputs["bq"], np.float32),
        np.asarray(inputs["Wk"], np.float32),
        np.asarray(inputs["bk"], np.float32),
        np.asarray(inputs["Wv"], np.float32),
        np.asarray(inputs["bv"], np.float32),
    )
    br = run_bass_kernel_spmd(_get_nc(), in_maps, list(range(N_CORES)), trace=trace)
    return _assemble(br.results), br


def kernel(**inputs) -> np.ndarray:
    out, _ = run(inputs)
    return out


# revision 23
# speedup vs baseline: 1.0011x; 1.0011x over previous
"""Trainium2 Bass kernel for nn_MultiHeadAttention (B=2, C=1024, H=16, S=2048).

Sharding: 8 cores = 2 batches x 4 head-groups (4 heads per core).
Per core:
  - QKV projections for its 4 heads (column-sharded weights), bias folded in
    via a ones-row appended to the inputs (K=1025 contraction).
  - Scores computed TRANSPOSED (scoresT[j,i] = k.q) so softmax normalization
    can ride along the ctx matmul: vT is augmented with a ones column so the
    ctx PSUM accumulates both sum_j P[j,i]*v[j,d] and sum_j P[j,i] (denom).
  - exp on ACT (no max-subtract needed: |scores| <= ~3), mask applied
    multiplicatively AFTER exp (exp(-1e9) == 0 equivalence).
  - Host does the final divide by the denominator row + transpose/concat.
"""

import numpy as np
import ml_dtypes

import concourse.bass as bass
import concourse.mybir as mybir
import concourse.tile as tile
from concourse import bacc
from concourse.bass_utils import run_bass_kernel_spmd

B = 2
C = 1024
HEADS = 16
CPH = 64
S = 2048
N_CORES = 8
HPC = 4  # heads per core
CPC = HPC * CPH  # channels per core = 256

BF = mybir.dt.bfloat16
F32 = mybir.dt.float32
F32R = mybir.dt.float32r
EXP = mybir.ActivationFunctionType.Exp

NBF = ml_dtypes.bfloat16

_NC_CACHE = {}


def build_nc():
    nc = bacc.Bacc("TRN2", target_bir_lowering=False)

    Qd = nc.declare_dram_parameter("Qin", [C, S], BF, isOutput=False)
    Kd = nc.declare_dram_parameter("Kin", [C, S], BF, isOutput=False)
    Vd = nc.declare_dram_parameter("Vin", [C, S], BF, isOutput=False)
    WqTd = nc.declare_dram_parameter("WqT", [C, CPC], BF, isOutput=False)
    WkTd = nc.declare_dram_parameter("WkT", [C, CPC], BF, isOutput=False)
    WvTd = nc.declare_dram_parameter("WvT", [C, HPC * 65], BF, isOutput=False)
    bqkd = nc.declare_dram_parameter("bqk", [128, 4], F32, isOutput=False)
    bvbd = nc.declare_dram_parameter("bvb", [128, HPC * 65], F32, isOutput=False)
    Md = nc.declare_dram_parameter("maskT", [S, S], BF, isOutput=False)
    Od = nc.declare_dram_parameter("out", [HPC * 65, S], F32, isOutput=True)

    with tile.TileContext(nc) as tc:
        with (
            tc.tile_pool(name="w", bufs=1) as wp,
            tc.tile_pool(name="qksb", bufs=1) as qkp,
            tc.tile_pool(name="vt", bufs=1) as vtp,
        ):
            # --- persistent SBUF tensors ---
            WqT = wp.tile([128, 8, CPC], BF, tag="wq")
            WkT = wp.tile([128, 8, CPC], BF, tag="wk")
            WvT = wp.tile([128, 8, HPC * 65], BF, tag="wv")
            bqk = wp.tile([128, 4], F32, tag="bqk")
            bvb = wp.tile([128, HPC * 65], F32, tag="bvb")
            for wt, wd in ((WqT, WqTd), (WkT, WkTd), (WvT, WvTd)):
                nc.sync.dma_start(
                    wt[:], wd[:].rearrange("(t p) n -> p t n", p=128)
                )
            nc.sync.dma_start(bqk[:], bqkd[:])
            nc.sync.dma_start(bvb[:], bvbd[:])

            q_sb = qkp.tile([128, 2, S], F32R, tag="q")  # pair-major, h-even rows 0:64
            k_sb = qkp.tile([128, 2, S], F32R, tag="k")
            vT = vtp.tile([128, 16, HPC * 65], BF, tag="vt")  # s_tile-major

            # --- phase 1: load inputs, projections (qk and vT interleaved) ---
            with tc.tile_pool(name="io", bufs=1) as io:
                Qin = io.tile([128, 8, S], BF, tag="qi")
                Kin = io.tile([128, 8, S], BF, tag="ki")
                Vin = io.tile([128, 8, S], BF, tag="vi")
                # V first (vT proj is first PE work), then Q/K interleaved
                for ci in range(8):
                    nc.sync.dma_start(Vin[:, ci, :], Vd[bass.ts(ci, 128), :])
                for ci in range(8):
                    nc.sync.dma_start(Qin[:, ci, :], Qd[bass.ts(ci, 128), :])
                    nc.sync.dma_start(Kin[:, ci, :], Kd[bass.ts(ci, 128), :])

                with (
                    tc.tile_pool(name="pp", bufs=6, space="PSUM") as pp,
                    tc.tile_pool(name="pv", bufs=2, space="PSUM") as pv,
                ):
                    # q/k projections per pair p, in i-halves (2-bank psum):
                    # psum rows 0:64 = head 2p, 64:128 = head 2p+1
                    # vT projection: vT[s, e] = sum_c V[c, s] * WvT[c, e]
                    for s in range(16):
                        ps = pv.tile([128, HPC * 65], F32, tag="pv")
                        for ci in range(8):
                            nc.tensor.matmul(
                                ps[:],
                                lhsT=Vin[:, ci, bass.ts(s, 128)],
                                rhs=WvT[:, ci, :],
                                start=(ci == 0),
                                stop=(ci == 7),
                            )
                        nc.vector.tensor_add(vT[:, s, :], ps[:], bvb[:])

                    for p in range(2):
                        for qk, (dst, wt, src) in enumerate(
                            ((q_sb, WqT, Qin), (k_sb, WkT, Kin))
                        ):
                            for n4 in range(4):
                                ps = pp.tile([128, 512], F32, tag="pp")
                                for ci in range(8):
                                    nc.tensor.matmul(
                                        ps[:],
                                        lhsT=wt[:, ci, bass.ts(p, 128)],
                                        rhs=src[:, ci, bass.ts(n4, 512)],
                                        start=(ci == 0),
                                        stop=(ci == 7),
                                    )
                                nc.scalar.add(
                                    dst[:, p, bass.ts(n4, 512)],
                                    ps[:],
                                    bqk[:, 2 * p + qk : 2 * p + qk + 1],
                                )

            # --- phase 3: attention ---
            with (
                tc.tile_pool(name="msk", bufs=1) as mkp,
                tc.tile_pool(name="pt", bufs=6) as ptp,
                tc.tile_pool(name="ob", bufs=4) as obp,
                tc.tile_pool(name="sc", bufs=2, space="PSUM") as scp,
                tc.tile_pool(name="cx", bufs=2, space="PSUM") as cxp,
            ):
                maskT = mkp.tile([128, 16, S], BF, tag="m")
                for j in range(16):
                    nc.sync.dma_start(
                        maskT[:, j, :], Md[bass.ts(j, 128), :]
                    )

                for p in range(2):
                    for hf in range(2):
                        cx = [
                            cxp.tile([65, 2, 512], F32, tag="cx", name=f"cx{i}")
                            for i in range(2)
                        ]
                        for j in range(16):
                            # both heads' score MMs emitted adjacently so the
                            # PE runs them concurrently (row groups 0-1 / 2-3)
                            sc0 = scp.tile([128, 1024], F32, tag="sc")
                            sc1 = scp.tile([128, 1024], F32, tag="sc")
                            for ib in range(2):
                                for hh, sc in ((0, sc0), (1, sc1)):
                                    lo, hi = 64 * hh, 64 * hh + 64
                                    nc.tensor.matmul(
                                        sc[:, bass.ts(ib, 512)],
                                        lhsT=k_sb[lo:hi, p, bass.ts(j, 128)],
                                        rhs=q_sb[
                                            lo:hi,
                                            p,
                                            bass.ds(hf * 1024 + ib * 512, 512),
                                        ],
                                        start=True,
                                        stop=True,
                                    )
                            pts = []
                            for hh, sc in ((0, sc0), (1, sc1)):
                                pt = ptp.tile([128, 1024], BF, tag="pt")
                                nc.scalar.activation(pt[:], sc[:], EXP)
                                nc.vector.tensor_mul(
                                    pt[:], pt[:], maskT[:, j, bass.ts(hf, 1024)]
                                )
                                pts.append(pt)
                            for hh in range(2):
                                hloc = 2 * p + hh
                                for ib in range(2):
                                    nc.tensor.matmul(
                                        cx[hh][:, ib, :],
                                        lhsT=vT[:, j, bass.ds(hloc * 65, 65)],
                                        rhs=pts[hh][:, bass.ts(ib, 512)],
                                        start=(j == 0),
                                        stop=(j == 15),
                                    )
                        # drain: one copy on DVE, one on ACT so the cx
                        # banks release in ~1us instead of 2 serial copies
                        for hh in range(2):
                            hloc = 2 * p + hh
                            ob = obp.tile([65, 2, 512], F32, tag="ob")
                            if hh == 0:
                                nc.vector.tensor_copy(ob[:], cx[hh][:])
                            else:
                                nc.scalar.copy(ob[:], cx[hh][:])
                            nc.sync.dma_start(
                                Od[
                                    bass.ds(hloc * 65, 65), bass.ts(hf, 1024)
                                ].rearrange("p (x y) -> p x y", x=2),
                                ob[:],
                            )
    nc.compile()
    return nc


def _get_nc():
    if "nc" not in _NC_CACHE:
        _NC_CACHE["nc"] = build_nc()
    return _NC_CACHE["nc"]


def _make_in_maps(Q, K, V, mask, Wq, bq, Wk, bk, Wv, bv):
    per_batch = []
    for b in range(B):
        Qa = Q[b].astype(NBF)
        Ka = K[b].astype(NBF)
        Va = V[b].astype(NBF)
        mT = np.ascontiguousarray((~mask[b]).T).astype(np.float32).astype(NBF)
        per_batch.append((Qa, Ka, Va, mT))

    in_maps = []
    for c in range(N_CORES):
        b, g = divmod(c, 4)
        hs = slice(g * CPC, (g + 1) * CPC)
        Qa, Ka, Va, mT = per_batch[b]
        WqTa = np.ascontiguousarray(Wq[hs].T / 8.0).astype(NBF)
        WkTa = np.ascontiguousarray(Wk[hs].T).astype(NBF)
        WvTa = np.zeros((C, HPC * 65), np.float32)
        bvba = np.zeros((128, HPC * 65), np.float32)
        for hh in range(HPC):
            ch = slice((g * HPC + hh) * CPH, (g * HPC + hh + 1) * CPH)
            WvTa[:, hh * 65 : hh * 65 + 64] = Wv[ch].T
            bvba[:, hh * 65 : hh * 65 + 64] = bv[ch][None, :]
            bvba[:, hh * 65 + 64] = 1.0
        # bias for q/k psum->sbuf copies: col 2p+qk = per-partition bias of
        # pair p's 128 channels (rows 0:64 = head 2p, 64:128 = head 2p+1)
        bqka = np.zeros((128, 4), np.float32)
        for p in range(2):
            ch = slice((g * 2 + p) * 128, (g * 2 + p + 1) * 128)
            bqka[:, 2 * p] = bq[ch] / 8.0
            bqka[:, 2 * p + 1] = bk[ch]
        in_maps.append(
            {
                "Qin": Qa,
                "Kin": Ka,
                "Vin": Va,
                "WqT": WqTa,
                "WkT": WkTa,
                "WvT": WvTa.astype(NBF),
                "bqk": bqka,
                "bvb": bvba,
                "maskT": mT,
            }
        )
    return in_maps


def _assemble(results):
    out = np.zeros((B, S, C), np.float32)
    for c in range(N_CORES):
        b, g = divmod(c, 4)
        o = results[c]["out"]  # [260, 2048]
        for hh in range(HPC):
            ctx = o[hh * 65 : hh * 65 + 64]  # [64, S] = (d, i)
            den = o[hh * 65 + 64]  # [S]
            ch0 = (g * HPC + hh) * CPH
            out[b, :, ch0 : ch0 + CPH] = (ctx / den[None, :]).T
    return out


def run(inputs, trace=False):
    in_maps = _make_in_maps(
        np.asarray(inputs["Q"], np.float32),
        np.asarray(inputs["K"], np.float32),
        np.asarray(inputs["V"], np.float32),
        np.asarray(inputs["mask"]),
        np.asarray(inputs["Wq"], np.float32),
        np.asarray(inputs["bq"], np.float32),
        np.asarray(inputs["Wk"], np.float32),
        np.asarray(inputs["bk"], np.float32),
        np.asarray(inputs["Wv"], np.float32),
        np.asarray(inputs["bv"], np.float32),
    )
    br = run_bass_kernel_spmd(_get_nc(), in_maps, list(range(N_CORES)), trace=trace)
    return _assemble(br.results), br


def kernel(**inputs) -> np.ndarray:
    out, _ = run(inputs)
    return out
